# revision 21
# baseline (speedup 1.0000x reference)
"""GRU decoder with tied-embedding projection on 8 Trainium2 NeuronCores.

Problem: B=32, T=256, H=1024, V=32000 (fp32).
    h_t = GRUCell(x_t, h_{t-1});  scores_t = h_t @ emb_w.T;  x_{t+1} = emb_w[gold_t]

Sharding: vocab-parallel (column-parallel tied projection). Every core runs the
(cheap, serial) GRU recurrence redundantly; each core computes a V/8 = 4000-wide
slice of the logits. No collectives; host concatenates the vocab slices.

Host-side input prep: the teacher-forced inputs are gathered (X = emb_w[idx])
and projected into gate space (GI = X @ w_ih.T + biases) on the host — GI is a
pure function of the inputs, independent of the recurrent state, and is laid
out per-step so the device reads it as a streamed operand.

Per-core kernel (all matmuls bf16, fp32 PSUM accumulation):
  - The recurrence matmul gh = h @ w_hh.T has only B=32 output rows, so it uses
    4-way PE *column tiling*: column group j computes a (permuted) 768-wide gate
    slice into PSUM partitions [32j, 32j+32).
  - Gate permutation P: group j holds [r,z,n] gates of hidden units
    [256j, 256j+256), so all gate math is partition-local.
  - gi_rz / the n-gate hidden bias are injected into PSUM with identity
    matmuls (PE is the only cross-partition data path), so the sigmoid reads
    PSUM directly.
  - All r/z MMs are emitted before the n MMs so the sigmoid's operand is ready
    at 2/3 of the recurrence span; gate math runs in bf16 (2x DVE modes).
  - h'^T is produced with identity-rhs matmuls straight into the chunk's HT
    tile, which doubles as the projection's stationary operand (no scatter).
  - Projection of chunk c-1 is interleaved between each step's recurrence MMs
    and the h-transpose, sized to cover the serial gate-math latency so the PE
    never idles (keeps the HAM clock-gate warm).
  - Logits are written in bf16 (host upcasts); well inside the error budget.
"""

import sys

import numpy as np

try:
    import concourse.bass as bass  # noqa: F401
except ImportError:  # grading env may not have it on sys.path
    sys.path.insert(0, "/opt/trn_rl_repo")

import concourse.bass as bass
import concourse.tile as tile
from concourse import mybir
from concourse.bass_utils import run_bass_kernel_spmd

import ml_dtypes

BF16 = mybir.dt.bfloat16
F32 = mybir.dt.float32
AF = mybir.ActivationFunctionType
ALU = mybir.AluOpType

N_CORES = 8
B = 32
H = 1024
NK = H // 128   # 8 k-tiles over the hidden dim
G3 = 3 * H      # 3072 gates
POS = [4 * (k % 2) + k // 2 for k in range(NK)]  # hid-block k -> HT col block


def _split_multi_waits(nc, limit=1):
    """Walrus (CoreV3, public build) accepts at most `limit` sem waits per
    instruction; move extra waits onto NoOps inserted just before."""
    n_new = 0
    for _name, bbw in nc.bb_map.items():
        insts = bbw.bb.instructions
        out, changed = [], False
        for inst in insts:
            si = inst.sync_info
            ws = list(si.on_wait) if si is not None else []
            if len(ws) > limit:
                changed = True
                for i in range(limit, len(ws), limit):
                    n_new += 1
                    nop = mybir.InstNoOp(
                        name=f"I-wsplit-{n_new}", engine=inst.engine, ins=[], outs=[]
                    )
                    nop.sync_info = mybir.SyncInfo(on_wait=ws[i : i + limit], on_update=[])
                    out.append(nop)
                inst.sync_info = mybir.SyncInfo(
                    on_wait=ws[:limit], on_update=list(si.on_update)
                )
            out.append(inst)
        if changed:
            bbw.bb.instructions = out
    return n_new


def _kblock(a):
    """[H, X] -> [128, NK*X]  (k-tile k occupies columns [k*X, (k+1)*X))."""
    hh, x = a.shape
    assert hh == H
    return np.ascontiguousarray(a.reshape(NK, 128, x).transpose(1, 0, 2).reshape(128, NK * x))


def _bf16(a):
    return np.asarray(a, dtype=ml_dtypes.bfloat16)


def build_program(T, Vs, Tc, PPS=2):
    """Build the SPMD bass program (identical on all cores)."""
    assert T % Tc == 0
    NCH = T // Tc            # chunks
    NV = Vs // 500           # 500-wide vocab chunks
    NM = (Tc * B) // 128     # projection m-tiles per chunk
    NT = 128 // B            # steps per projection m-tile (m covers tl in [NT*m, NT*m+NT))

    nc = bass.Bass()
    d_whh = nc.declare_dram_parameter("whhp", [128, NK * G3], BF16, isOutput=False)
    d_emb = nc.declare_dram_parameter("embc", [128, NK * Vs], BF16, isOutput=False)
    d_gis = nc.declare_dram_parameter("gis", [T, 128, 768], BF16, isOutput=False)
    d_bhn = nc.declare_dram_parameter("bhnp", [128, 256], BF16, isOutput=False)
    d_i128 = nc.declare_dram_parameter("i128", [128, 128], BF16, isOutput=False)
    d_h0b = nc.declare_dram_parameter("h0b", [128, 256], BF16, isOutput=False)
    d_h0t = nc.declare_dram_parameter("h0t", [128, 256], BF16, isOutput=False)
    d_out = nc.declare_dram_parameter("scores", [B, T, Vs], BF16, isOutput=True)

    with tile.TileContext(nc) as tc:
        with (
            tc.tile_pool(name="consts", bufs=1) as consts,
            tc.tile_pool(name="gic", bufs=2) as p_gi,
            tc.tile_pool(name="htc", bufs=3) as p_ht,
            tc.tile_pool(name="rz", bufs=2) as p_rz,
            tc.tile_pool(name="t12", bufs=2) as p_t12,
            tc.tile_pool(name="nh", bufs=2) as p_nh,
            tc.tile_pool(name="hdz", bufs=2) as p_hdz,
            tc.tile_pool(name="hb", bufs=2) as p_hb,
            tc.tile_pool(name="pstage", bufs=4) as p_stage,
            tc.tile_pool(name="psrz", bufs=2, space="PSUM") as ps_rz,
            tc.tile_pool(name="psn", bufs=2, space="PSUM") as ps_n,
            tc.tile_pool(name="psht", bufs=1, space="PSUM") as ps_t,
            tc.tile_pool(name="pspr", bufs=3, space="PSUM") as ps_pr,
        ):
            whh = consts.tile([128, NK * G3], BF16, tag="whh")
            nc.sync.dma_start(whh[:], d_whh[:])
            i128 = consts.tile([128, 128], BF16, tag="i128")
            nc.sync.dma_start(i128[:], d_i128[:])
            bhn = consts.tile([128, 256], BF16, tag="bhn")
            nc.sync.dma_start(bhn[:], d_bhn[:])
            h0b = consts.tile([128, 256], BF16, tag="h0b")
            nc.sync.dma_start(h0b[:], d_h0b[:])
            h0t = consts.tile([128, 256], BF16, tag="h0t")
            nc.sync.dma_start(h0t[:], d_h0t[:])
            emb = consts.tile([128, NK * Vs], BF16, tag="emb")
            nc.sync.dma_start(emb[:], d_emb[:])

            gi_tiles = {}
            gi_tiles[0] = p_gi.tile([128, Tc, 768], BF16, tag="gic", name="gi_c0")
            nc.sync.dma_start(
                gi_tiles[0][:], d_gis[0:Tc].rearrange("t p g -> p t g")
            )

            h_prev = h0b[:]            # [32j+b, 256]  gate-math layout
            ht_prev = lambda k: h0t[:, 32 * POS[k] : 32 * POS[k] + 32]
            proj_queue = []

            def emit_proj_unit(ci, ht_c, m, n):
                # HT chunk layout: col = pos*(Tc*32) + tl*32 + b; m-tile m is
                # rows (tl, b) for tl in [NT*m, NT*m+NT) -> contiguous lhsT.
                pp = ps_pr.tile([128, 500], F32, tag="pspr")
                for k in range(NK):
                    lhs = ht_c[:, POS[k], 32 * NT * m : 32 * NT * m + 128]
                    nc.tensor.matmul(
                        pp[:],
                        lhs,
                        emb[:, k * Vs + n * 500 : k * Vs + n * 500 + 500],
                        start=(k == 0),
                        stop=(k == NK - 1),
                    )
                st = p_stage.tile([128, 500], BF16, tag="pstage")
                if (m + n) % 2 == 0:
                    nc.scalar.copy(st[:], pp[:])
                else:
                    nc.vector.tensor_copy(st[:], pp[:])
                nc.sync.dma_start(
                    d_out[
                        :,
                        ci * Tc + NT * m : ci * Tc + NT * m + NT,
                        n * 500 : n * 500 + 500,
                    ].rearrange("b t v -> t b v"),
                    st[:],
                )

            for ci in range(NCH):
                gi_c = gi_tiles.pop(ci)
                if ci + 1 < NCH:  # prefetch next chunk's gi one chunk ahead
                    gi_tiles[ci + 1] = p_gi.tile(
                        [128, Tc, 768], BF16, tag="gic", name=f"gi_c{ci + 1}"
                    )
                    nc.sync.dma_start(
                        gi_tiles[ci + 1][:],
                        d_gis[(ci + 1) * Tc : (ci + 2) * Tc].rearrange("t p g -> p t g"),
                    )
                ht_c = p_ht.tile([128, NK, Tc * 32], BF16, tag="htc")
                for tl in range(Tc):
                    gh_rz = ps_rz.tile([128, 512], F32, tag="psrz")
                    gh_n = ps_n.tile([128, 256], F32, tag="psn")
                    # bias/gi injection (no h dependency; PE fills these early)
                    nc.tensor.matmul(
                        gh_rz[:], i128[:], gi_c[:, tl, 0:512], start=True, stop=False
                    )
                    nc.tensor.matmul(
                        gh_n[:], i128[:], bhn[:], start=True, stop=False
                    )
                    # r/z recurrence MMs first (sigmoid operand ready earliest)
                    for k in range(NK):
                        lhs = ht_prev(k)
                        for j in range(4):
                            nc.tensor.matmul(
                                gh_rz[32 * j : 32 * j + 32, :],
                                lhs,
                                whh[:, k * G3 + 768 * j : k * G3 + 768 * j + 512],
                                start=False,
                                stop=(k == NK - 1),
                                tile_position=(0, 32 * j),
                            )
                    # n MMs (overlap the sigmoid)
                    for k in range(NK):
                        lhs = ht_prev(k)
                        for j in range(4):
                            nc.tensor.matmul(
                                gh_n[32 * j : 32 * j + 32, :],
                                lhs,
                                whh[:, k * G3 + 768 * j + 512 : k * G3 + 768 * j + 768],
                                start=False,
                                stop=(k == NK - 1),
                                tile_position=(0, 32 * j),
                            )
                    # ---- gate math (bf16, partition-local) ----
                    rz = p_rz.tile([128, 512], BF16, tag="rz")
                    nc.scalar.activation(rz[:, 0:256], gh_rz[:, 0:256], AF.Sigmoid)
                    nc.scalar.activation(rz[:, 256:512], gh_rz[:, 256:512], AF.Sigmoid)
                    t12 = p_t12.tile([128, 512], BF16, tag="t12")
                    nc.vector.tensor_tensor(t12[:, 0:256], rz[:, 0:256], gh_n[:], ALU.mult)
                    nc.vector.tensor_tensor(
                        t12[:, 256:512], t12[:, 0:256], gi_c[:, tl, 512:768], ALU.add
                    )
                    nh = p_nh.tile([128, 256], BF16, tag="nh")
                    nc.scalar.activation(nh[:], t12[:, 256:512], AF.Tanh)
                    hdz = p_hdz.tile([128, 512], BF16, tag="hdz")
                    nc.vector.tensor_tensor(hdz[:, 0:256], h_prev, nh[:], ALU.subtract)
                    nc.vector.tensor_tensor(
                        hdz[:, 256:512], rz[:, 256:512], hdz[:, 0:256], ALU.mult
                    )
                    h_new = p_hb.tile([128, 256], BF16, tag="hb")
                    nc.vector.tensor_tensor(h_new[:], nh[:], hdz[:, 256:512], ALU.add)
                    # projection filler under the gate-math dependency chain
                    for _ in range(min(PPS, len(proj_queue))):
                        emit_proj_unit(*proj_queue.pop(0))
                    # ---- h'^T via identity-rhs matmuls, straight into HT ----
                    pT = ps_t.tile([128, 256], F32, tag="psht")
                    nc.tensor.matmul(
                        pT[:, 0:128], h_new[:, 0:128], i128[:], start=True, stop=True
                    )
                    nc.tensor.matmul(
                        pT[:, 128:256], h_new[:, 128:256], i128[:], start=True, stop=True
                    )
                    nc.vector.tensor_copy(
                        ht_c[:, :, tl * 32 : tl * 32 + 32],
                        pT[:].rearrange("p (h j b) -> p (h j) b", h=2, j=4),
                    )
                    ht_prev = (
                        lambda k, tl=tl, ht_c=ht_c: ht_c[
                            :, POS[k], tl * 32 : tl * 32 + 32
                        ]
                    )
                    h_prev = h_new[:]
                for m in range(NM):
                    for n in range(NV):
                        proj_queue.append((ci, ht_c, m, n))
            while proj_queue:
                emit_proj_unit(*proj_queue.pop(0))

    nc.finalize()
    _split_multi_waits(nc)
    return nc


def _gate_perm():
    P = np.empty(G3, np.int64)
    for j in range(4):
        u = np.arange(256) + 256 * j
        P[768 * j : 768 * j + 256] = u
        P[768 * j + 256 : 768 * j + 512] = H + u
        P[768 * j + 512 : 768 * j + 768] = 2 * H + u
    return P


def prep_inputs(enc_hiddens, emb_w, w_ih, w_hh, b_ih, b_hh, gold, T, Vs, n_cores):
    """Host-side shard + layout prep. Returns per-core input maps."""
    h0 = np.asarray(enc_hiddens, np.float32)[0]          # [B, H]
    emb_w = np.asarray(emb_w, np.float32)
    w_ih = np.asarray(w_ih, np.float32)
    w_hh = np.asarray(w_hh, np.float32)
    b_ih = np.asarray(b_ih, np.float32)
    b_hh = np.asarray(b_hh, np.float32)
    gold = np.asarray(gold)

    P = _gate_perm()
    whhp = _bf16(_kblock(w_hh[P].T))

    # teacher-forced inputs -> gate space (host prep; state-independent)
    idx = np.empty((T, B), np.int64)
    idx[0] = 1  # START_IDX
    if T > 1:
        idx[1:] = gold[:, : T - 1].T
    X = emb_w[idx].reshape(T * B, H)                      # [T*B, H]
    mask = (np.arange(G3) < 2 * H).astype(np.float32)
    gib_row = b_ih + b_hh * mask                          # rz biases summed; n: b_ih only
    GI = X @ w_ih.T + gib_row                             # [T*B, 3H] fp32
    gis = _bf16(
        np.ascontiguousarray(
            GI.reshape(T, B, 3, 4, 256).transpose(0, 3, 1, 2, 4).reshape(T, 128, 768)
        )
    )
    bhnp = _bf16(np.repeat(b_hh[2 * H :].reshape(4, 256), 32, axis=0))
    i128 = _bf16(np.eye(128, dtype=np.float32))
    h0b = _bf16(h0.reshape(B, 4, 256).transpose(1, 0, 2).reshape(128, 256))
    h0t = _bf16(
        np.ascontiguousarray(h0.reshape(B, 4, 2, 128).transpose(3, 2, 1, 0).reshape(128, 256))
    )
    embT = emb_w.T                                        # [H, V]
    maps = []
    for c in range(n_cores):
        embc = _bf16(_kblock(np.ascontiguousarray(embT[:, c * Vs : (c + 1) * Vs])))
        maps.append(
            dict(whhp=whhp, embc=embc, gis=gis, bhnp=bhnp, i128=i128, h0b=h0b, h0t=h0t)
        )
    return maps


_CACHE = {}


def run(enc_hiddens, emb_w, w_ih, w_hh, b_ih, b_hh, gold, T=256, Vs=4000,
        n_cores=8, Tc=8, trace=False):
    key = (T, Vs, n_cores, Tc)
    if key not in _CACHE:
        _CACHE[key] = build_program(T, Vs, Tc)
    nc = _CACHE[key]
    maps = prep_inputs(enc_hiddens, emb_w, w_ih, w_hh, b_ih, b_hh, gold, T, Vs, n_cores)
    res = run_bass_kernel_spmd(nc, maps, list(range(n_cores)), trace=trace)
    out = np.concatenate(
        [np.asarray(res.results[c]["scores"], np.float32) for c in range(n_cores)],
        axis=2,
    )
    return out, res


def kernel(enc_hiddens, emb_w, w_ih, w_hh, b_ih, b_hh, gold):
    out, _ = run(enc_hiddens, emb_w, w_ih, w_hh, b_ih, b_hh, gold)
    return out


# revision 22
# speedup vs baseline: 1.0364x; 1.0364x over previous
"""GRU decoder with tied-embedding projection on 8 Trainium2 NeuronCores.

Problem: B=32, T=256, H=1024, V=32000 (fp32).
    h_t = GRUCell(x_t, h_{t-1});  scores_t = h_t @ emb_w.T;  x_{t+1} = emb_w[gold_t]

Sharding: vocab-parallel (column-parallel tied projection). Every core runs the
(cheap, serial) GRU recurrence redundantly; each core computes a V/8 = 4000-wide
slice of the logits. No collectives; host concatenates the vocab slices.

Host-side input prep: the teacher-forced inputs are gathered (X = emb_w[idx])
and projected into gate space (GI = X @ w_ih.T + biases) on the host — GI is a
pure function of the inputs, independent of the recurrent state, and is laid
out per-step so the device reads it as a streamed operand.

Per-core kernel (all matmuls bf16, fp32 PSUM accumulation):
  - The recurrence matmul gh = h @ w_hh.T has only B=32 output rows, so it uses
    4-way PE *column tiling*: column group j computes a (permuted) 768-wide gate
    slice into PSUM partitions [32j, 32j+32).
  - Gate permutation P: group j holds [r,z,n] gates of hidden units
    [256j, 256j+256), so all gate math is partition-local.
  - gi_rz / the n-gate hidden bias are injected into PSUM with identity
    matmuls (PE is the only cross-partition data path), so the sigmoid reads
    PSUM directly.
  - All r/z MMs are emitted before the n MMs so the sigmoid's operand is ready
    at 2/3 of the recurrence span; gate math runs in bf16 (2x DVE modes).
  - h'^T is produced with identity-rhs matmuls straight into the chunk's HT
    tile, which doubles as the projection's stationary operand (no scatter).
  - Projection of chunk c-1 is interleaved between each step's recurrence MMs
    and the h-transpose, sized to cover the serial gate-math latency so the PE
    never idles (keeps the HAM clock-gate warm).
  - Logits are written in bf16 (host upcasts); well inside the error budget.
"""

import sys

import numpy as np

try:
    import concourse.bass as bass  # noqa: F401
except ImportError:  # grading env may not have it on sys.path
    sys.path.insert(0, "/opt/trn_rl_repo")

import concourse.bass as bass
import concourse.tile as tile
from concourse import mybir
from concourse.bass_utils import run_bass_kernel_spmd

import ml_dtypes

BF16 = mybir.dt.bfloat16
F32 = mybir.dt.float32
AF = mybir.ActivationFunctionType
ALU = mybir.AluOpType

N_CORES = 8
B = 32
H = 1024
NK = H // 128   # 8 k-tiles over the hidden dim
G3 = 3 * H      # 3072 gates
POS = [4 * (k % 2) + k // 2 for k in range(NK)]  # hid-block k -> HT col block


def _split_multi_waits(nc, limit=1):
    """Walrus (CoreV3, public build) accepts at most `limit` sem waits per
    instruction; move extra waits onto NoOps inserted just before."""
    n_new = 0
    for _name, bbw in nc.bb_map.items():
        insts = bbw.bb.instructions
        out, changed = [], False
        for inst in insts:
            si = inst.sync_info
            ws = list(si.on_wait) if si is not None else []
            if len(ws) > limit:
                changed = True
                for i in range(limit, len(ws), limit):
                    n_new += 1
                    nop = mybir.InstNoOp(
                        name=f"I-wsplit-{n_new}", engine=inst.engine, ins=[], outs=[]
                    )
                    nop.sync_info = mybir.SyncInfo(on_wait=ws[i : i + limit], on_update=[])
                    out.append(nop)
                inst.sync_info = mybir.SyncInfo(
                    on_wait=ws[:limit], on_update=list(si.on_update)
                )
            out.append(inst)
        if changed:
            bbw.bb.instructions = out
    return n_new


def _kblock(a):
    """[H, X] -> [128, NK*X]  (k-tile k occupies columns [k*X, (k+1)*X))."""
    hh, x = a.shape
    assert hh == H
    return np.ascontiguousarray(a.reshape(NK, 128, x).transpose(1, 0, 2).reshape(128, NK * x))


def _bf16(a):
    return np.asarray(a, dtype=ml_dtypes.bfloat16)


def build_program(T, Vs, Tc, PPS=2):
    """Build the SPMD bass program (identical on all cores)."""
    assert T % Tc == 0
    NCH = T // Tc            # chunks
    NV = Vs // 500           # 500-wide vocab chunks
    NM = (Tc * B) // 128     # projection m-tiles per chunk
    NT = 128 // B            # steps per projection m-tile (m covers tl in [NT*m, NT*m+NT))

    nc = bass.Bass()
    d_whh = nc.declare_dram_parameter("whhp", [128, NK * G3], BF16, isOutput=False)
    d_emb = nc.declare_dram_parameter("embc", [128, NK * Vs], BF16, isOutput=False)
    d_gis = nc.declare_dram_parameter("gis", [T, 128, 768], BF16, isOutput=False)
    d_bhn = nc.declare_dram_parameter("bhnp", [128, 256], BF16, isOutput=False)
    d_i128 = nc.declare_dram_parameter("i128", [128, 128], BF16, isOutput=False)
    d_h0b = nc.declare_dram_parameter("h0b", [128, 256], BF16, isOutput=False)
    d_h0t = nc.declare_dram_parameter("h0t", [128, 256], BF16, isOutput=False)
    d_out = nc.declare_dram_parameter("scores", [B, T, Vs], BF16, isOutput=True)

    with tile.TileContext(nc) as tc:
        with (
            tc.tile_pool(name="consts", bufs=1) as consts,
            tc.tile_pool(name="gic", bufs=2) as p_gi,
            tc.tile_pool(name="htc", bufs=3) as p_ht,
            tc.tile_pool(name="rz", bufs=2) as p_rz,
            tc.tile_pool(name="t12", bufs=2) as p_t12,
            tc.tile_pool(name="nh", bufs=2) as p_nh,
            tc.tile_pool(name="hdz", bufs=2) as p_hdz,
            tc.tile_pool(name="hb", bufs=2) as p_hb,
            tc.tile_pool(name="pstage", bufs=4) as p_stage,
            tc.tile_pool(name="psrz", bufs=2, space="PSUM") as ps_rz,
            tc.tile_pool(name="psn", bufs=2, space="PSUM") as ps_n,
            tc.tile_pool(name="psht", bufs=1, space="PSUM") as ps_t,
            tc.tile_pool(name="pspr", bufs=3, space="PSUM") as ps_pr,
        ):
            whh = consts.tile([128, NK * G3], BF16, tag="whh")
            nc.sync.dma_start(whh[:], d_whh[:])
            i128 = consts.tile([128, 128], BF16, tag="i128")
            nc.sync.dma_start(i128[:], d_i128[:])
            bhn = consts.tile([128, 256], BF16, tag="bhn")
            nc.sync.dma_start(bhn[:], d_bhn[:])
            h0b = consts.tile([128, 256], BF16, tag="h0b")
            nc.sync.dma_start(h0b[:], d_h0b[:])
            h0t = consts.tile([128, 256], BF16, tag="h0t")
            nc.sync.dma_start(h0t[:], d_h0t[:])
            emb = consts.tile([128, NK * Vs], BF16, tag="emb")
            nc.sync.dma_start(emb[:], d_emb[:])

            h_prev = h0b[:]            # [32j+b, 256]  gate-math layout
            ht_prev = lambda k: h0t[:, 32 * POS[k] : 32 * POS[k] + 32]
            proj_queue = []

            def emit_proj_unit(ci, ht_c, m, n):
                # HT chunk layout: col = pos*(Tc*32) + tl*32 + b; m-tile m is
                # rows (tl, b) for tl in [NT*m, NT*m+NT) -> contiguous lhsT.
                pp = ps_pr.tile([128, 500], F32, tag="pspr")
                for k in range(NK):
                    lhs = ht_c[:, POS[k], 32 * NT * m : 32 * NT * m + 128]
                    nc.tensor.matmul(
                        pp[:],
                        lhs,
                        emb[:, k * Vs + n * 500 : k * Vs + n * 500 + 500],
                        start=(k == 0),
                        stop=(k == NK - 1),
                    )
                st = p_stage.tile([128, 500], BF16, tag="pstage")
                if (m + n) % 2 == 0:
                    nc.scalar.copy(st[:], pp[:])
                else:
                    nc.vector.tensor_copy(st[:], pp[:])
                nc.sync.dma_start(
                    d_out[
                        :,
                        ci * Tc + NT * m : ci * Tc + NT * m + NT,
                        n * 500 : n * 500 + 500,
                    ].rearrange("b t v -> t b v"),
                    st[:],
                )

            for ci in range(NCH):
                gi_c = p_gi.tile([128, Tc, 768], BF16, tag="gic")
                # split: step-0 slice lands fast so the chunk's first inject
                # isn't gated on the full 1.6MB transfer
                nc.sync.dma_start(
                    gi_c[:, 0:1, :],
                    d_gis[ci * Tc : ci * Tc + 1].rearrange("t p g -> p t g"),
                )
                nc.sync.dma_start(
                    gi_c[:, 1:Tc, :],
                    d_gis[ci * Tc + 1 : ci * Tc + Tc].rearrange("t p g -> p t g"),
                )
                ht_c = p_ht.tile([128, NK, Tc * 32], BF16, tag="htc")
                for tl in range(Tc):
                    gh_rz = ps_rz.tile([128, 512], F32, tag="psrz")
                    gh_n = ps_n.tile([128, 256], F32, tag="psn")
                    # bias/gi injection (no h dependency; PE fills these early)
                    nc.tensor.matmul(
                        gh_rz[:], i128[:], gi_c[:, tl, 0:512], start=True, stop=False
                    )
                    nc.tensor.matmul(
                        gh_n[:], i128[:], bhn[:], start=True, stop=False
                    )
                    # r/z recurrence MMs first (sigmoid operand ready earliest)
                    for k in range(NK):
                        lhs = ht_prev(k)
                        for j in range(4):
                            nc.tensor.matmul(
                                gh_rz[32 * j : 32 * j + 32, :],
                                lhs,
                                whh[:, k * G3 + 768 * j : k * G3 + 768 * j + 512],
                                start=False,
                                stop=(k == NK - 1),
                                tile_position=(0, 32 * j),
                            )
                    # n MMs (overlap the sigmoid)
                    for k in range(NK):
                        lhs = ht_prev(k)
                        for j in range(4):
                            nc.tensor.matmul(
                                gh_n[32 * j : 32 * j + 32, :],
                                lhs,
                                whh[:, k * G3 + 768 * j + 512 : k * G3 + 768 * j + 768],
                                start=False,
                                stop=(k == NK - 1),
                                tile_position=(0, 32 * j),
                            )
                    # ---- gate math (bf16, partition-local) ----
                    rz = p_rz.tile([128, 512], BF16, tag="rz")
                    nc.scalar.activation(rz[:, 0:256], gh_rz[:, 0:256], AF.Sigmoid)
                    nc.scalar.activation(rz[:, 256:512], gh_rz[:, 256:512], AF.Sigmoid)
                    t12 = p_t12.tile([128, 512], BF16, tag="t12")
                    nc.vector.tensor_tensor(t12[:, 0:256], rz[:, 0:256], gh_n[:], ALU.mult)
                    nc.vector.tensor_tensor(
                        t12[:, 256:512], t12[:, 0:256], gi_c[:, tl, 512:768], ALU.add
                    )
                    nh = p_nh.tile([128, 256], BF16, tag="nh")
                    nc.scalar.activation(nh[:], t12[:, 256:512], AF.Tanh)
                    hdz = p_hdz.tile([128, 512], BF16, tag="hdz")
                    nc.vector.tensor_tensor(hdz[:, 0:256], h_prev, nh[:], ALU.subtract)
                    nc.vector.tensor_tensor(
                        hdz[:, 256:512], rz[:, 256:512], hdz[:, 0:256], ALU.mult
                    )
                    h_new = p_hb.tile([128, 256], BF16, tag="hb")
                    nc.vector.tensor_tensor(h_new[:], nh[:], hdz[:, 256:512], ALU.add)
                    # projection filler under the gate-math dependency chain
                    for _ in range(min(PPS, len(proj_queue))):
                        emit_proj_unit(*proj_queue.pop(0))
                    # ---- h'^T via identity-rhs matmuls, straight into HT ----
                    pT = ps_t.tile([128, 256], F32, tag="psht")
                    nc.tensor.matmul(
                        pT[:, 0:128], h_new[:, 0:128], i128[:], start=True, stop=True
                    )
                    nc.tensor.matmul(
                        pT[:, 128:256], h_new[:, 128:256], i128[:], start=True, stop=True
                    )
                    nc.vector.tensor_copy(
                        ht_c[:, :, tl * 32 : tl * 32 + 32],
                        pT[:].rearrange("p (h j b) -> p (h j) b", h=2, j=4),
                    )
                    ht_prev = (
                        lambda k, tl=tl, ht_c=ht_c: ht_c[
                            :, POS[k], tl * 32 : tl * 32 + 32
                        ]
                    )
                    h_prev = h_new[:]
                for m in range(NM):
                    for n in range(NV):
                        proj_queue.append((ci, ht_c, m, n))
            while proj_queue:
                emit_proj_unit(*proj_queue.pop(0))

    nc.finalize()
    _split_multi_waits(nc)
    return nc


def _gate_perm():
    P = np.empty(G3, np.int64)
    for j in range(4):
        u = np.arange(256) + 256 * j
        P[768 * j : 768 * j + 256] = u
        P[768 * j + 256 : 768 * j + 512] = H + u
        P[768 * j + 512 : 768 * j + 768] = 2 * H + u
    return P


def prep_inputs(enc_hiddens, emb_w, w_ih, w_hh, b_ih, b_hh, gold, T, Vs, n_cores):
    """Host-side shard + layout prep. Returns per-core input maps."""
    h0 = np.asarray(enc_hiddens, np.float32)[0]          # [B, H]
    emb_w = np.asarray(emb_w, np.float32)
    w_ih = np.asarray(w_ih, np.float32)
    w_hh = np.asarray(w_hh, np.float32)
    b_ih = np.asarray(b_ih, np.float32)
    b_hh = np.asarray(b_hh, np.float32)
    gold = np.asarray(gold)

    P = _gate_perm()
    whhp = _bf16(_kblock(w_hh[P].T))

    # teacher-forced inputs -> gate space (host prep; state-independent)
    idx = np.empty((T, B), np.int64)
    idx[0] = 1  # START_IDX
    if T > 1:
        idx[1:] = gold[:, : T - 1].T
    X = emb_w[idx].reshape(T * B, H)                      # [T*B, H]
    mask = (np.arange(G3) < 2 * H).astype(np.float32)
    gib_row = b_ih + b_hh * mask                          # rz biases summed; n: b_ih only
    GI = X @ w_ih.T + gib_row                             # [T*B, 3H] fp32
    gis = _bf16(
        np.ascontiguousarray(
            GI.reshape(T, B, 3, 4, 256).transpose(0, 3, 1, 2, 4).reshape(T, 128, 768)
        )
    )
    bhnp = _bf16(np.repeat(b_hh[2 * H :].reshape(4, 256), 32, axis=0))
    i128 = _bf16(np.eye(128, dtype=np.float32))
    h0b = _bf16(h0.reshape(B, 4, 256).transpose(1, 0, 2).reshape(128, 256))
    h0t = _bf16(
        np.ascontiguousarray(h0.reshape(B, 4, 2, 128).transpose(3, 2, 1, 0).reshape(128, 256))
    )
    embT = emb_w.T                                        # [H, V]
    maps = []
    for c in range(n_cores):
        embc = _bf16(_kblock(np.ascontiguousarray(embT[:, c * Vs : (c + 1) * Vs])))
        maps.append(
            dict(whhp=whhp, embc=embc, gis=gis, bhnp=bhnp, i128=i128, h0b=h0b, h0t=h0t)
        )
    return maps


_CACHE = {}


def run(enc_hiddens, emb_w, w_ih, w_hh, b_ih, b_hh, gold, T=256, Vs=4000,
        n_cores=8, Tc=8, trace=False):
    key = (T, Vs, n_cores, Tc)
    if key not in _CACHE:
        _CACHE[key] = build_program(T, Vs, Tc)
    nc = _CACHE[key]
    maps = prep_inputs(enc_hiddens, emb_w, w_ih, w_hh, b_ih, b_hh, gold, T, Vs, n_cores)
    res = run_bass_kernel_spmd(nc, maps, list(range(n_cores)), trace=trace)
    out = np.concatenate(
        [np.asarray(res.results[c]["scores"], np.float32) for c in range(n_cores)],
        axis=2,
    )
    return out, res


def kernel(enc_hiddens, emb_w, w_ih, w_hh, b_ih, b_hh, gold):
    out, _ = run(enc_hiddens, emb_w, w_ih, w_hh, b_ih, b_hh, gold)
    return out


# revision 23
# speedup vs baseline: 1.0696x; 1.0321x over previous
"""GRU decoder with tied-embedding projection on 8 Trainium2 NeuronCores.

Problem: B=32, T=256, H=1024, V=32000 (fp32).
    h_t = GRUCell(x_t, h_{t-1});  scores_t = h_t @ emb_w.T;  x_{t+1} = emb_w[gold_t]

Sharding: vocab-parallel (column-parallel tied projection). Every core runs the
(cheap, serial) GRU recurrence redundantly; each core computes a V/8 = 4000-wide
slice of the logits. No collectives; host concatenates the vocab slices.

Host-side input prep: the teacher-forced inputs are gathered (X = emb_w[idx])
and projected into gate space (GI = X @ w_ih.T + biases) on the host — GI is a
pure function of the inputs, independent of the recurrent state, and is laid
out per-step so the device reads it as a streamed operand.

Per-core kernel (all matmuls bf16, fp32 PSUM accumulation):
  - The recurrence matmul gh = h @ w_hh.T has only B=32 output rows, so it uses
    4-way PE *column tiling*: column group j computes a (permuted) 768-wide gate
    slice into PSUM partitions [32j, 32j+32).
  - Gate permutation P: group j holds [r,z,n] gates of hidden units
    [256j, 256j+256), so all gate math is partition-local.
  - gi_rz / the n-gate hidden bias are injected into PSUM with identity
    matmuls (PE is the only cross-partition data path), so the sigmoid reads
    PSUM directly.
  - All r/z MMs are emitted before the n MMs so the sigmoid's operand is ready
    at 2/3 of the recurrence span; gate math runs in bf16 (2x DVE modes).
  - h'^T is produced with identity-rhs matmuls straight into the chunk's HT
    tile, which doubles as the projection's stationary operand (no scatter).
  - Projection of chunk c-1 is interleaved between each step's recurrence MMs
    and the h-transpose, sized to cover the serial gate-math latency so the PE
    never idles (keeps the HAM clock-gate warm).
  - Logits are written in bf16 (host upcasts); well inside the error budget.
"""

import sys

import numpy as np

try:
    import concourse.bass as bass  # noqa: F401
except ImportError:  # grading env may not have it on sys.path
    sys.path.insert(0, "/opt/trn_rl_repo")

import concourse.bass as bass
import concourse.tile as tile
from concourse import mybir
from concourse.bass_utils import run_bass_kernel_spmd

import ml_dtypes

BF16 = mybir.dt.bfloat16
F32 = mybir.dt.float32
AF = mybir.ActivationFunctionType
ALU = mybir.AluOpType

N_CORES = 8
B = 32
H = 1024
NK = H // 128   # 8 k-tiles over the hidden dim
G3 = 3 * H      # 3072 gates
POS = [4 * (k % 2) + k // 2 for k in range(NK)]  # hid-block k -> HT col block


def _split_multi_waits(nc, limit=1):
    """Walrus (CoreV3, public build) accepts at most `limit` sem waits per
    instruction; move extra waits onto NoOps inserted just before."""
    n_new = 0
    for _name, bbw in nc.bb_map.items():
        insts = bbw.bb.instructions
        out, changed = [], False
        for inst in insts:
            si = inst.sync_info
            ws = list(si.on_wait) if si is not None else []
            if len(ws) > limit:
                changed = True
                for i in range(limit, len(ws), limit):
                    n_new += 1
                    nop = mybir.InstNoOp(
                        name=f"I-wsplit-{n_new}", engine=inst.engine, ins=[], outs=[]
                    )
                    nop.sync_info = mybir.SyncInfo(on_wait=ws[i : i + limit], on_update=[])
                    out.append(nop)
                inst.sync_info = mybir.SyncInfo(
                    on_wait=ws[:limit], on_update=list(si.on_update)
                )
            out.append(inst)
        if changed:
            bbw.bb.instructions = out
    return n_new


def _kblock(a):
    """[H, X] -> [128, NK*X]  (k-tile k occupies columns [k*X, (k+1)*X))."""
    hh, x = a.shape
    assert hh == H
    return np.ascontiguousarray(a.reshape(NK, 128, x).transpose(1, 0, 2).reshape(128, NK * x))


def _bf16(a):
    return np.asarray(a, dtype=ml_dtypes.bfloat16)


def build_program(T, Vs, Tc, PPS=2):
    """Build the SPMD bass program (identical on all cores)."""
    assert T % Tc == 0
    NCH = T // Tc            # chunks
    NV = Vs // 500           # 500-wide vocab chunks
    NM = (Tc * B) // 128     # projection m-tiles per chunk
    NT = 128 // B            # steps per projection m-tile (m covers tl in [NT*m, NT*m+NT))

    nc = bass.Bass()
    d_whh = nc.declare_dram_parameter("whhp", [128, NK * G3], BF16, isOutput=False)
    d_emb = nc.declare_dram_parameter("embc", [128, NK * Vs], BF16, isOutput=False)
    d_gis = nc.declare_dram_parameter("gis", [T, 128, 768], BF16, isOutput=False)
    d_bhn = nc.declare_dram_parameter("bhnp", [128, 256], BF16, isOutput=False)
    d_i128 = nc.declare_dram_parameter("i128", [128, 128], BF16, isOutput=False)
    d_h0b = nc.declare_dram_parameter("h0b", [128, 256], BF16, isOutput=False)
    d_h0t = nc.declare_dram_parameter("h0t", [128, 256], BF16, isOutput=False)
    d_out = nc.declare_dram_parameter("scores", [B, T, Vs], BF16, isOutput=True)

    with tile.TileContext(nc) as tc:
        with (
            tc.tile_pool(name="consts", bufs=1) as consts,
            tc.tile_pool(name="gic", bufs=2) as p_gi,
            tc.tile_pool(name="htc", bufs=3) as p_ht,
            tc.tile_pool(name="rz", bufs=2) as p_rz,
            tc.tile_pool(name="t12", bufs=2) as p_t12,
            tc.tile_pool(name="nh", bufs=2) as p_nh,
            tc.tile_pool(name="hdz", bufs=2) as p_hdz,
            tc.tile_pool(name="hb", bufs=2) as p_hb,
            tc.tile_pool(name="pstage", bufs=4) as p_stage,
            tc.tile_pool(name="psrz", bufs=2, space="PSUM") as ps_rz,
            tc.tile_pool(name="psn", bufs=2, space="PSUM") as ps_n,
            tc.tile_pool(name="psht", bufs=1, space="PSUM") as ps_t,
            tc.tile_pool(name="pspr", bufs=3, space="PSUM") as ps_pr,
        ):
            whh = consts.tile([128, NK * G3], BF16, tag="whh")
            nc.sync.dma_start(whh[:], d_whh[:])
            i128 = consts.tile([128, 128], BF16, tag="i128")
            nc.sync.dma_start(i128[:], d_i128[:])
            bhn = consts.tile([128, 256], BF16, tag="bhn")
            nc.sync.dma_start(bhn[:], d_bhn[:])
            h0b = consts.tile([128, 256], BF16, tag="h0b")
            nc.sync.dma_start(h0b[:], d_h0b[:])
            h0t = consts.tile([128, 256], BF16, tag="h0t")
            nc.sync.dma_start(h0t[:], d_h0t[:])
            emb = consts.tile([128, NK * Vs], BF16, tag="emb")
            nc.sync.dma_start(emb[:], d_emb[:])

            h_prev = h0b[:]            # [32j+b, 256]  gate-math layout
            ht_prev = lambda k: h0t[:, 32 * POS[k] : 32 * POS[k] + 32]
            proj_queue = []

            def emit_proj_unit(ci, ht_c, m, n):
                # HT chunk layout: col = pos*(Tc*32) + tl*32 + b; m-tile m is
                # rows (tl, b) for tl in [NT*m, NT*m+NT) -> contiguous lhsT.
                pp = ps_pr.tile([128, 500], F32, tag="pspr")
                for k in range(NK):
                    lhs = ht_c[:, POS[k], 32 * NT * m : 32 * NT * m + 128]
                    nc.tensor.matmul(
                        pp[:],
                        lhs,
                        emb[:, k * Vs + n * 500 : k * Vs + n * 500 + 500],
                        start=(k == 0),
                        stop=(k == NK - 1),
                    )
                st = p_stage.tile([128, 500], BF16, tag="pstage")
                if (m + n) % 2 == 0:
                    nc.scalar.copy(st[:], pp[:])
                else:
                    nc.vector.tensor_copy(st[:], pp[:])
                nc.sync.dma_start(
                    d_out[
                        :,
                        ci * Tc + NT * m : ci * Tc + NT * m + NT,
                        n * 500 : n * 500 + 500,
                    ].rearrange("b t v -> t b v"),
                    st[:],
                )

            for ci in range(NCH):
                gi_c = p_gi.tile([128, Tc, 768], BF16, tag="gic")
                nc.sync.dma_start(
                    gi_c[:], d_gis[ci * Tc : ci * Tc + Tc].rearrange("t p g -> p t g")
                )
                ht_c = p_ht.tile([128, NK, Tc * 32], BF16, tag="htc")
                for tl in range(Tc):
                    gh_rz = ps_rz.tile([128, 512], F32, tag="psrz")
                    gh_n = ps_n.tile([128, 256], F32, tag="psn")
                    # bias/gi injection (no h dependency; PE fills these early)
                    nc.tensor.matmul(
                        gh_rz[:], i128[:], gi_c[:, tl, 0:512], start=True, stop=False
                    )
                    nc.tensor.matmul(
                        gh_n[:], i128[:], bhn[:], start=True, stop=False
                    )
                    # r/z recurrence MMs first (sigmoid operand ready earliest)
                    for k in range(NK):
                        lhs = ht_prev(k)
                        for j in range(4):
                            nc.tensor.matmul(
                                gh_rz[32 * j : 32 * j + 32, :],
                                lhs,
                                whh[:, k * G3 + 768 * j : k * G3 + 768 * j + 512],
                                start=False,
                                stop=(k == NK - 1),
                                tile_position=(0, 32 * j),
                            )
                    # n MMs (overlap the sigmoid)
                    for k in range(NK):
                        lhs = ht_prev(k)
                        for j in range(4):
                            nc.tensor.matmul(
                                gh_n[32 * j : 32 * j + 32, :],
                                lhs,
                                whh[:, k * G3 + 768 * j + 512 : k * G3 + 768 * j + 768],
                                start=False,
                                stop=(k == NK - 1),
                                tile_position=(0, 32 * j),
                            )
                    # ---- gate math (bf16, partition-local) ----
                    rz = p_rz.tile([128, 512], BF16, tag="rz")
                    nc.scalar.activation(rz[:, 0:256], gh_rz[:, 0:256], AF.Sigmoid)
                    nc.scalar.activation(rz[:, 256:512], gh_rz[:, 256:512], AF.Sigmoid)
                    t12 = p_t12.tile([128, 512], BF16, tag="t12")
                    nc.vector.tensor_tensor(t12[:, 0:256], rz[:, 0:256], gh_n[:], ALU.mult)
                    nc.vector.tensor_tensor(
                        t12[:, 256:512], t12[:, 0:256], gi_c[:, tl, 512:768], ALU.add
                    )
                    nh = p_nh.tile([128, 256], BF16, tag="nh")
                    nc.scalar.activation(nh[:], t12[:, 256:512], AF.Tanh)
                    hdz = p_hdz.tile([128, 512], BF16, tag="hdz")
                    nc.vector.tensor_tensor(hdz[:, 0:256], h_prev, nh[:], ALU.subtract)
                    nc.vector.tensor_tensor(
                        hdz[:, 256:512], rz[:, 256:512], hdz[:, 0:256], ALU.mult
                    )
                    h_new = p_hb.tile([128, 256], BF16, tag="hb")
                    nc.vector.tensor_tensor(h_new[:], nh[:], hdz[:, 256:512], ALU.add)
                    # projection filler under the gate-math dependency chain
                    for _ in range(min(PPS, len(proj_queue))):
                        emit_proj_unit(*proj_queue.pop(0))
                    # ---- h'^T via identity-rhs matmuls, straight into HT ----
                    pT = ps_t.tile([128, 256], F32, tag="psht")
                    nc.tensor.matmul(
                        pT[:, 0:128], h_new[:, 0:128], i128[:], start=True, stop=True
                    )
                    nc.tensor.matmul(
                        pT[:, 128:256], h_new[:, 128:256], i128[:], start=True, stop=True
                    )
                    nc.vector.tensor_copy(
                        ht_c[:, :, tl * 32 : tl * 32 + 32],
                        pT[:].rearrange("p (h j b) -> p (h j) b", h=2, j=4),
                    )
                    ht_prev = (
                        lambda k, tl=tl, ht_c=ht_c: ht_c[
                            :, POS[k], tl * 32 : tl * 32 + 32
                        ]
                    )
                    h_prev = h_new[:]
                for m in range(NM):
                    for n in range(NV):
                        proj_queue.append((ci, ht_c, m, n))
            while proj_queue:
                emit_proj_unit(*proj_queue.pop(0))

    nc.finalize()
    _split_multi_waits(nc)
    return nc


def _gate_perm():
    P = np.empty(G3, np.int64)
    for j in range(4):
        u = np.arange(256) + 256 * j
        P[768 * j : 768 * j + 256] = u
        P[768 * j + 256 : 768 * j + 512] = H + u
        P[768 * j + 512 : 768 * j + 768] = 2 * H + u
    return P


def prep_inputs(enc_hiddens, emb_w, w_ih, w_hh, b_ih, b_hh, gold, T, Vs, n_cores):
    """Host-side shard + layout prep. Returns per-core input maps."""
    h0 = np.asarray(enc_hiddens, np.float32)[0]          # [B, H]
    emb_w = np.asarray(emb_w, np.float32)
    w_ih = np.asarray(w_ih, np.float32)
    w_hh = np.asarray(w_hh, np.float32)
    b_ih = np.asarray(b_ih, np.float32)
    b_hh = np.asarray(b_hh, np.float32)
    gold = np.asarray(gold)

    P = _gate_perm()
    whhp = _bf16(_kblock(w_hh[P].T))

    # teacher-forced inputs -> gate space (host prep; state-independent)
    idx = np.empty((T, B), np.int64)
    idx[0] = 1  # START_IDX
    if T > 1:
        idx[1:] = gold[:, : T - 1].T
    X = emb_w[idx].reshape(T * B, H)                      # [T*B, H]
    mask = (np.arange(G3) < 2 * H).astype(np.float32)
    gib_row = b_ih + b_hh * mask                          # rz biases summed; n: b_ih only
    GI = X @ w_ih.T + gib_row                             # [T*B, 3H] fp32
    gis = _bf16(
        np.ascontiguousarray(
            GI.reshape(T, B, 3, 4, 256).transpose(0, 3, 1, 2, 4).reshape(T, 128, 768)
        )
    )
    bhnp = _bf16(np.repeat(b_hh[2 * H :].reshape(4, 256), 32, axis=0))
    i128 = _bf16(np.eye(128, dtype=np.float32))
    h0b = _bf16(h0.reshape(B, 4, 256).transpose(1, 0, 2).reshape(128, 256))
    h0t = _bf16(
        np.ascontiguousarray(h0.reshape(B, 4, 2, 128).transpose(3, 2, 1, 0).reshape(128, 256))
    )
    embT = emb_w.T                                        # [H, V]
    maps = []
    for c in range(n_cores):
        embc = _bf16(_kblock(np.ascontiguousarray(embT[:, c * Vs : (c + 1) * Vs])))
        maps.append(
            dict(whhp=whhp, embc=embc, gis=gis, bhnp=bhnp, i128=i128, h0b=h0b, h0t=h0t)
        )
    return maps


_CACHE = {}


def run(enc_hiddens, emb_w, w_ih, w_hh, b_ih, b_hh, gold, T=256, Vs=4000,
        n_cores=8, Tc=16, trace=False):
    key = (T, Vs, n_cores, Tc)
    if key not in _CACHE:
        _CACHE[key] = build_program(T, Vs, Tc)
    nc = _CACHE[key]
    maps = prep_inputs(enc_hiddens, emb_w, w_ih, w_hh, b_ih, b_hh, gold, T, Vs, n_cores)
    res = run_bass_kernel_spmd(nc, maps, list(range(n_cores)), trace=trace)
    out = np.concatenate(
        [np.asarray(res.results[c]["scores"], np.float32) for c in range(n_cores)],
        axis=2,
    )
    return out, res


def kernel(enc_hiddens, emb_w, w_ih, w_hh, b_ih, b_hh, gold):
    out, _ = run(enc_hiddens, emb_w, w_ih, w_hh, b_ih, b_hh, gold)
    return out


# revision 24
# speedup vs baseline: 1.1508x; 1.0759x over previous
"""GRU decoder with tied-embedding projection on 8 Trainium2 NeuronCores.

Problem: B=32, T=256, H=1024, V=32000 (fp32).
    h_t = GRUCell(x_t, h_{t-1});  scores_t = h_t @ emb_w.T;  x_{t+1} = emb_w[gold_t]

Sharding: vocab-parallel (column-parallel tied projection). Every core runs the
(cheap, serial) GRU recurrence redundantly; each core computes a V/8 = 4000-wide
slice of the logits. No collectives; host concatenates the vocab slices.

Host-side input prep: the teacher-forced inputs are gathered (X = emb_w[idx])
and projected into gate space (GI = X @ w_ih.T + biases) on the host — GI is a
pure function of the inputs, independent of the recurrent state, and is laid
out per-step so the device reads it as a streamed operand.

Per-core kernel (all matmuls bf16, fp32 PSUM accumulation):
  - The recurrence matmul gh = h @ w_hh.T has only B=32 output rows, so it uses
    4-way PE *column tiling*: column group j computes a (permuted) 768-wide gate
    slice into PSUM partitions [32j, 32j+32).
  - Gate permutation P: group j holds [r,z,n] gates of hidden units
    [256j, 256j+256), so all gate math is partition-local.
  - gi_rz / the n-gate hidden bias are injected into PSUM with identity
    matmuls (PE is the only cross-partition data path), so the sigmoid reads
    PSUM directly.
  - All r/z MMs are emitted before the n MMs so the sigmoid's operand is ready
    at 2/3 of the recurrence span; gate math runs in bf16 (2x DVE modes).
  - h'^T is produced with identity-rhs matmuls straight into the chunk's HT
    tile, which doubles as the projection's stationary operand (no scatter).
  - Projection of chunk c-1 is interleaved between each step's recurrence MMs
    and the h-transpose, sized to cover the serial gate-math latency so the PE
    never idles (keeps the HAM clock-gate warm).
  - Logits are written in bf16 (host upcasts); well inside the error budget.
"""

import sys

import numpy as np

try:
    import concourse.bass as bass  # noqa: F401
except ImportError:  # grading env may not have it on sys.path
    sys.path.insert(0, "/opt/trn_rl_repo")

import concourse.bass as bass
import concourse.tile as tile
from concourse import mybir
from concourse.bass_utils import run_bass_kernel_spmd

import ml_dtypes

BF16 = mybir.dt.bfloat16
F32 = mybir.dt.float32
AF = mybir.ActivationFunctionType
ALU = mybir.AluOpType

N_CORES = 8
B = 32
H = 1024
NK = H // 128   # 8 k-tiles over the hidden dim
G3 = 3 * H      # 3072 gates
POS = [4 * (k % 2) + k // 2 for k in range(NK)]  # hid-block k -> HT col block


def _split_multi_waits(nc, limit=1):
    """Walrus (CoreV3, public build) accepts at most `limit` sem waits per
    instruction; move extra waits onto NoOps inserted just before."""
    n_new = 0
    for _name, bbw in nc.bb_map.items():
        insts = bbw.bb.instructions
        out, changed = [], False
        for inst in insts:
            si = inst.sync_info
            ws = list(si.on_wait) if si is not None else []
            if len(ws) > limit:
                changed = True
                for i in range(limit, len(ws), limit):
                    n_new += 1
                    nop = mybir.InstNoOp(
                        name=f"I-wsplit-{n_new}", engine=inst.engine, ins=[], outs=[]
                    )
                    nop.sync_info = mybir.SyncInfo(on_wait=ws[i : i + limit], on_update=[])
                    out.append(nop)
                inst.sync_info = mybir.SyncInfo(
                    on_wait=ws[:limit], on_update=list(si.on_update)
                )
            out.append(inst)
        if changed:
            bbw.bb.instructions = out
    return n_new


def _kblock(a):
    """[H, X] -> [128, NK*X]  (k-tile k occupies columns [k*X, (k+1)*X))."""
    hh, x = a.shape
    assert hh == H
    return np.ascontiguousarray(a.reshape(NK, 128, x).transpose(1, 0, 2).reshape(128, NK * x))


def _bf16(a):
    return np.asarray(a, dtype=ml_dtypes.bfloat16)


def build_program(T, Vs, Tc, PPS=2):
    """Build the SPMD bass program (identical on all cores)."""
    assert T % Tc == 0
    NCH = T // Tc            # chunks
    NV = Vs // 500           # 500-wide vocab chunks
    NM = (Tc * B) // 128     # projection m-tiles per chunk
    NT = 128 // B            # steps per projection m-tile (m covers tl in [NT*m, NT*m+NT))

    nc = bass.Bass()
    d_whh = nc.declare_dram_parameter("whhp", [128, NK * G3], BF16, isOutput=False)
    d_emb = nc.declare_dram_parameter("embc", [128, NK * Vs], BF16, isOutput=False)
    d_gis = nc.declare_dram_parameter("gis", [T, 128, 768], BF16, isOutput=False)
    d_bhn = nc.declare_dram_parameter("bhnp", [128, 256], BF16, isOutput=False)
    d_i128 = nc.declare_dram_parameter("i128", [128, 128], BF16, isOutput=False)
    d_h0b = nc.declare_dram_parameter("h0b", [128, 256], BF16, isOutput=False)
    d_h0t = nc.declare_dram_parameter("h0t", [128, 256], BF16, isOutput=False)
    d_out = nc.declare_dram_parameter("scores", [B, T, Vs], BF16, isOutput=True)

    with tile.TileContext(nc) as tc:
        with (
            tc.tile_pool(name="consts", bufs=1) as consts,
            tc.tile_pool(name="gic", bufs=2) as p_gi,
            tc.tile_pool(name="htc", bufs=3) as p_ht,
            tc.tile_pool(name="rz", bufs=2) as p_rz,
            tc.tile_pool(name="t12", bufs=2) as p_t12,
            tc.tile_pool(name="nh", bufs=2) as p_nh,
            tc.tile_pool(name="hdz", bufs=2) as p_hdz,
            tc.tile_pool(name="hb", bufs=2) as p_hb,
            tc.tile_pool(name="pstage", bufs=4) as p_stage,
            tc.tile_pool(name="psrz", bufs=2, space="PSUM") as ps_rz,
            tc.tile_pool(name="psn", bufs=2, space="PSUM") as ps_n,
            tc.tile_pool(name="psht", bufs=1, space="PSUM") as ps_t,
            tc.tile_pool(name="pspr", bufs=3, space="PSUM") as ps_pr,
        ):
            whh = consts.tile([128, NK * G3], BF16, tag="whh")
            nc.sync.dma_start(whh[:], d_whh[:])
            i128 = consts.tile([128, 128], BF16, tag="i128")
            nc.sync.dma_start(i128[:], d_i128[:])
            bhn = consts.tile([128, 256], BF16, tag="bhn")
            nc.sync.dma_start(bhn[:], d_bhn[:])
            h0b = consts.tile([128, 256], BF16, tag="h0b")
            nc.sync.dma_start(h0b[:], d_h0b[:])
            h0t = consts.tile([128, 256], BF16, tag="h0t")
            nc.sync.dma_start(h0t[:], d_h0t[:])
            emb = consts.tile([128, NK * Vs], BF16, tag="emb")
            nc.sync.dma_start(emb[:], d_emb[:])

            h_prev = h0b[:]            # [32j+b, 256]  gate-math layout
            ht_prev = lambda k: h0t[:, 32 * POS[k] : 32 * POS[k] + 32]
            proj_queue = []

            def emit_proj_unit(ci, ht_c, m, n):
                # HT chunk layout: col = pos*(Tc*32) + tl*32 + b; m-tile m is
                # rows (tl, b) for tl in [NT*m, NT*m+NT) -> contiguous lhsT.
                pp = ps_pr.tile([128, 500], F32, tag="pspr")
                for k in range(NK):
                    lhs = ht_c[:, POS[k], 32 * NT * m : 32 * NT * m + 128]
                    nc.tensor.matmul(
                        pp[:],
                        lhs,
                        emb[:, k * Vs + n * 500 : k * Vs + n * 500 + 500],
                        start=(k == 0),
                        stop=(k == NK - 1),
                    )
                st = p_stage.tile([128, 500], BF16, tag="pstage")
                if (m + n) % 2 == 0:
                    nc.scalar.copy(st[:], pp[:])
                else:
                    nc.vector.tensor_copy(st[:], pp[:])
                nc.sync.dma_start(
                    d_out[
                        :,
                        ci * Tc + NT * m : ci * Tc + NT * m + NT,
                        n * 500 : n * 500 + 500,
                    ].rearrange("b t v -> t b v"),
                    st[:],
                )

            for ci in range(NCH):
                gi_c = p_gi.tile([128, Tc, 768], BF16, tag="gic")
                nc.sync.dma_start(
                    gi_c[:], d_gis[ci * Tc : ci * Tc + Tc].rearrange("t p g -> p t g")
                )
                ht_c = p_ht.tile([128, NK, Tc * 32], BF16, tag="htc")
                for tl in range(Tc):
                    gh_rz = ps_rz.tile([128, 512], F32, tag="psrz")
                    gh_n = ps_n.tile([128, 256], F32, tag="psn")
                    # bias/gi injection (no h dependency; PE fills these early)
                    nc.tensor.matmul(
                        gh_rz[:], i128[:], gi_c[:, tl, 0:512], start=True, stop=False
                    )
                    nc.tensor.matmul(
                        gh_n[:], i128[:], bhn[:], start=True, stop=False
                    )
                    # r/z recurrence MMs first (sigmoid operand ready earliest)
                    for k in range(NK):
                        lhs = ht_prev(k)
                        for j in range(4):
                            nc.tensor.matmul(
                                gh_rz[32 * j : 32 * j + 32, :],
                                lhs,
                                whh[:, k * G3 + 768 * j : k * G3 + 768 * j + 512],
                                start=False,
                                stop=(k == NK - 1),
                                tile_position=(0, 32 * j),
                            )
                    # n MMs (overlap the sigmoid)
                    for k in range(NK):
                        lhs = ht_prev(k)
                        for j in range(4):
                            nc.tensor.matmul(
                                gh_n[32 * j : 32 * j + 32, :],
                                lhs,
                                whh[:, k * G3 + 768 * j + 512 : k * G3 + 768 * j + 768],
                                start=False,
                                stop=(k == NK - 1),
                                tile_position=(0, 32 * j),
                            )
                    # ---- gate math (bf16, partition-local) ----
                    rz = p_rz.tile([128, 512], BF16, tag="rz")
                    nc.scalar.activation(rz[:, 0:256], gh_rz[:, 0:256], AF.Sigmoid)
                    nc.scalar.activation(rz[:, 256:512], gh_rz[:, 256:512], AF.Sigmoid)
                    t12 = p_t12.tile([128, 512], BF16, tag="t12")
                    nc.vector.tensor_tensor(t12[:, 0:256], rz[:, 0:256], gh_n[:], ALU.mult)
                    nc.vector.tensor_tensor(
                        t12[:, 256:512], t12[:, 0:256], gi_c[:, tl, 512:768], ALU.add
                    )
                    nh = p_nh.tile([128, 256], BF16, tag="nh")
                    nc.scalar.activation(nh[:], t12[:, 256:512], AF.Tanh)
                    hdz = p_hdz.tile([128, 512], BF16, tag="hdz")
                    nc.vector.tensor_tensor(hdz[:, 0:256], h_prev, nh[:], ALU.subtract)
                    nc.vector.tensor_tensor(
                        hdz[:, 256:512], rz[:, 256:512], hdz[:, 0:256], ALU.mult
                    )
                    h_new = p_hb.tile([128, 256], BF16, tag="hb")
                    nc.vector.tensor_tensor(h_new[:], nh[:], hdz[:, 256:512], ALU.add)
                    # projection filler under the gate-math dependency chain
                    for _ in range(min(PPS, len(proj_queue))):
                        emit_proj_unit(*proj_queue.pop(0))
                    # ---- h'^T via identity-rhs matmuls, straight into HT ----
                    pT = ps_t.tile([128, 256], F32, tag="psht")
                    nc.tensor.matmul(
                        pT[:, 0:128], h_new[:, 0:128], i128[:], start=True, stop=True
                    )
                    nc.tensor.matmul(
                        pT[:, 128:256], h_new[:, 128:256], i128[:], start=True, stop=True
                    )
                    nc.vector.tensor_copy(
                        ht_c[:, :, tl * 32 : tl * 32 + 32],
                        pT[:].rearrange("p (h j b) -> p (h j) b", h=2, j=4),
                    )
                    ht_prev = (
                        lambda k, tl=tl, ht_c=ht_c: ht_c[
                            :, POS[k], tl * 32 : tl * 32 + 32
                        ]
                    )
                    h_prev = h_new[:]
                    # queue m-tile (tl+1)/NT-1 as soon as its ht cols are
                    # written: supply lands every NT steps instead of 2*NM*NV
                    # units at once at the chunk boundary
                    if (tl + 1) % NT == 0:
                        m = (tl + 1) // NT - 1
                        for n in range(NV):
                            proj_queue.append((ci, ht_c, m, n))
            while proj_queue:
                emit_proj_unit(*proj_queue.pop(0))

    nc.finalize()
    _split_multi_waits(nc)
    return nc


def _gate_perm():
    P = np.empty(G3, np.int64)
    for j in range(4):
        u = np.arange(256) + 256 * j
        P[768 * j : 768 * j + 256] = u
        P[768 * j + 256 : 768 * j + 512] = H + u
        P[768 * j + 512 : 768 * j + 768] = 2 * H + u
    return P


def prep_inputs(enc_hiddens, emb_w, w_ih, w_hh, b_ih, b_hh, gold, T, Vs, n_cores):
    """Host-side shard + layout prep. Returns per-core input maps."""
    h0 = np.asarray(enc_hiddens, np.float32)[0]          # [B, H]
    emb_w = np.asarray(emb_w, np.float32)
    w_ih = np.asarray(w_ih, np.float32)
    w_hh = np.asarray(w_hh, np.float32)
    b_ih = np.asarray(b_ih, np.float32)
    b_hh = np.asarray(b_hh, np.float32)
    gold = np.asarray(gold)

    P = _gate_perm()
    whhp = _bf16(_kblock(w_hh[P].T))

    # teacher-forced inputs -> gate space (host prep; state-independent)
    idx = np.empty((T, B), np.int64)
    idx[0] = 1  # START_IDX
    if T > 1:
        idx[1:] = gold[:, : T - 1].T
    X = emb_w[idx].reshape(T * B, H)                      # [T*B, H]
    mask = (np.arange(G3) < 2 * H).astype(np.float32)
    gib_row = b_ih + b_hh * mask                          # rz biases summed; n: b_ih only
    GI = X @ w_ih.T + gib_row                             # [T*B, 3H] fp32
    gis = _bf16(
        np.ascontiguousarray(
            GI.reshape(T, B, 3, 4, 256).transpose(0, 3, 1, 2, 4).reshape(T, 128, 768)
        )
    )
    bhnp = _bf16(np.repeat(b_hh[2 * H :].reshape(4, 256), 32, axis=0))
    i128 = _bf16(np.eye(128, dtype=np.float32))
    h0b = _bf16(h0.reshape(B, 4, 256).transpose(1, 0, 2).reshape(128, 256))
    h0t = _bf16(
        np.ascontiguousarray(h0.reshape(B, 4, 2, 128).transpose(3, 2, 1, 0).reshape(128, 256))
    )
    embT = emb_w.T                                        # [H, V]
    maps = []
    for c in range(n_cores):
        embc = _bf16(_kblock(np.ascontiguousarray(embT[:, c * Vs : (c + 1) * Vs])))
        maps.append(
            dict(whhp=whhp, embc=embc, gis=gis, bhnp=bhnp, i128=i128, h0b=h0b, h0t=h0t)
        )
    return maps


_CACHE = {}


def run(enc_hiddens, emb_w, w_ih, w_hh, b_ih, b_hh, gold, T=256, Vs=4000,
        n_cores=8, Tc=8, trace=False):
    key = (T, Vs, n_cores, Tc)
    if key not in _CACHE:
        _CACHE[key] = build_program(T, Vs, Tc)
    nc = _CACHE[key]
    maps = prep_inputs(enc_hiddens, emb_w, w_ih, w_hh, b_ih, b_hh, gold, T, Vs, n_cores)
    res = run_bass_kernel_spmd(nc, maps, list(range(n_cores)), trace=trace)
    out = np.concatenate(
        [np.asarray(res.results[c]["scores"], np.float32) for c in range(n_cores)],
        axis=2,
    )
    return out, res


def kernel(enc_hiddens, emb_w, w_ih, w_hh, b_ih, b_hh, gold):
    out, _ = run(enc_hiddens, emb_w, w_ih, w_hh, b_ih, b_hh, gold)
    return out


# revision 25
# speedup vs baseline: 1.2092x; 1.0508x over previous
"""GRU decoder with tied-embedding projection on 8 Trainium2 NeuronCores.

Problem: B=32, T=256, H=1024, V=32000 (fp32).
    h_t = GRUCell(x_t, h_{t-1});  scores_t = h_t @ emb_w.T;  x_{t+1} = emb_w[gold_t]

Sharding: vocab-parallel (column-parallel tied projection). Every core runs the
(cheap, serial) GRU recurrence redundantly; each core computes a V/8 = 4000-wide
slice of the logits. No collectives; host concatenates the vocab slices.

Host-side input prep: the teacher-forced inputs are gathered (X = emb_w[idx])
and projected into gate space (GI = X @ w_ih.T + biases) on the host — GI is a
pure function of the inputs, independent of the recurrent state, and is laid
out per-step so the device reads it as a streamed operand.

Per-core kernel (all matmuls bf16, fp32 PSUM accumulation):
  - The recurrence matmul gh = h @ w_hh.T has only B=32 output rows, so it uses
    4-way PE *column tiling*: column group j computes a (permuted) 768-wide gate
    slice into PSUM partitions [32j, 32j+32).
  - Gate permutation P: group j holds [r,z,n] gates of hidden units
    [256j, 256j+256), so all gate math is partition-local.
  - gi_rz / the n-gate hidden bias are injected into PSUM with identity
    matmuls (PE is the only cross-partition data path), so the sigmoid reads
    PSUM directly.
  - All r/z MMs are emitted before the n MMs so the sigmoid's operand is ready
    at 2/3 of the recurrence span; gate math runs in bf16 (2x DVE modes).
  - h'^T is produced with identity-rhs matmuls straight into the chunk's HT
    tile, which doubles as the projection's stationary operand (no scatter).
  - Projection of chunk c-1 is interleaved between each step's recurrence MMs
    and the h-transpose, sized to cover the serial gate-math latency so the PE
    never idles (keeps the HAM clock-gate warm).
  - Logits are written in bf16 (host upcasts); well inside the error budget.
"""

import sys

import numpy as np

try:
    import concourse.bass as bass  # noqa: F401
except ImportError:  # grading env may not have it on sys.path
    sys.path.insert(0, "/opt/trn_rl_repo")

import concourse.bass as bass
import concourse.tile as tile
from concourse import mybir
from concourse.bass_utils import run_bass_kernel_spmd

import ml_dtypes

BF16 = mybir.dt.bfloat16
F32 = mybir.dt.float32
AF = mybir.ActivationFunctionType
ALU = mybir.AluOpType

N_CORES = 8
B = 32
H = 1024
NK = H // 128   # 8 k-tiles over the hidden dim
G3 = 3 * H      # 3072 gates
POS = [4 * (k % 2) + k // 2 for k in range(NK)]  # hid-block k -> HT col block


def _split_multi_waits(nc, limit=1):
    """Walrus (CoreV3, public build) accepts at most `limit` sem waits per
    instruction; move extra waits onto NoOps inserted just before."""
    n_new = 0
    for _name, bbw in nc.bb_map.items():
        insts = bbw.bb.instructions
        out, changed = [], False
        for inst in insts:
            si = inst.sync_info
            ws = list(si.on_wait) if si is not None else []
            if len(ws) > limit:
                changed = True
                for i in range(limit, len(ws), limit):
                    n_new += 1
                    nop = mybir.InstNoOp(
                        name=f"I-wsplit-{n_new}", engine=inst.engine, ins=[], outs=[]
                    )
                    nop.sync_info = mybir.SyncInfo(on_wait=ws[i : i + limit], on_update=[])
                    out.append(nop)
                inst.sync_info = mybir.SyncInfo(
                    on_wait=ws[:limit], on_update=list(si.on_update)
                )
            out.append(inst)
        if changed:
            bbw.bb.instructions = out
    return n_new


def _kblock(a):
    """[H, X] -> [128, NK*X]  (k-tile k occupies columns [k*X, (k+1)*X))."""
    hh, x = a.shape
    assert hh == H
    return np.ascontiguousarray(a.reshape(NK, 128, x).transpose(1, 0, 2).reshape(128, NK * x))


def _bf16(a):
    return np.asarray(a, dtype=ml_dtypes.bfloat16)


def build_program(T, Vs, Tc, PPS=2):
    """Build the SPMD bass program (identical on all cores)."""
    assert T % Tc == 0
    NCH = T // Tc            # chunks
    NV = Vs // 500           # 500-wide vocab chunks
    NM = (Tc * B) // 128     # projection m-tiles per chunk
    NT = 128 // B            # steps per projection m-tile (m covers tl in [NT*m, NT*m+NT))

    nc = bass.Bass()
    d_whh = nc.declare_dram_parameter("whhp", [128, NK * G3], BF16, isOutput=False)
    d_emb = nc.declare_dram_parameter("embc", [128, NK * Vs], BF16, isOutput=False)
    d_gis = nc.declare_dram_parameter("gis", [T, 128, 768], BF16, isOutput=False)
    d_bhn = nc.declare_dram_parameter("bhnp", [128, 256], BF16, isOutput=False)
    d_i128 = nc.declare_dram_parameter("i128", [128, 128], BF16, isOutput=False)
    d_h0b = nc.declare_dram_parameter("h0b", [128, 256], BF16, isOutput=False)
    d_h0t = nc.declare_dram_parameter("h0t", [128, 256], BF16, isOutput=False)
    d_out = nc.declare_dram_parameter("scores", [B, T, Vs], BF16, isOutput=True)

    with tile.TileContext(nc) as tc:
        with (
            tc.tile_pool(name="consts", bufs=1) as consts,
            tc.tile_pool(name="gic", bufs=2) as p_gi,
            tc.tile_pool(name="htc", bufs=3) as p_ht,
            tc.tile_pool(name="rz", bufs=2) as p_rz,
            tc.tile_pool(name="t12", bufs=2) as p_t12,
            tc.tile_pool(name="nh", bufs=2) as p_nh,
            tc.tile_pool(name="hdz", bufs=2) as p_hdz,
            tc.tile_pool(name="hb", bufs=2) as p_hb,
            tc.tile_pool(name="pstage", bufs=4) as p_stage,
            tc.tile_pool(name="psrz", bufs=2, space="PSUM") as ps_rz,
            tc.tile_pool(name="psn", bufs=1, space="PSUM") as ps_n,
            tc.tile_pool(name="psht", bufs=1, space="PSUM") as ps_t,
            tc.tile_pool(name="pspr", bufs=4, space="PSUM") as ps_pr,
        ):
            whh = consts.tile([128, NK * G3], BF16, tag="whh")
            nc.sync.dma_start(whh[:], d_whh[:])
            i128 = consts.tile([128, 128], BF16, tag="i128")
            nc.sync.dma_start(i128[:], d_i128[:])
            bhn = consts.tile([128, 256], BF16, tag="bhn")
            nc.sync.dma_start(bhn[:], d_bhn[:])
            h0b = consts.tile([128, 256], BF16, tag="h0b")
            nc.sync.dma_start(h0b[:], d_h0b[:])
            h0t = consts.tile([128, 256], BF16, tag="h0t")
            nc.sync.dma_start(h0t[:], d_h0t[:])
            emb = consts.tile([128, NK * Vs], BF16, tag="emb")
            nc.sync.dma_start(emb[:], d_emb[:])

            h_prev = h0b[:]            # [32j+b, 256]  gate-math layout
            ht_prev = lambda k: h0t[:, 32 * POS[k] : 32 * POS[k] + 32]
            proj_queue = []

            def emit_proj_unit(ci, ht_c, m, n):
                # HT chunk layout: col = pos*(Tc*32) + tl*32 + b; m-tile m is
                # rows (tl, b) for tl in [NT*m, NT*m+NT) -> contiguous lhsT.
                pp = ps_pr.tile([128, 500], F32, tag="pspr")
                for k in range(NK):
                    lhs = ht_c[:, POS[k], 32 * NT * m : 32 * NT * m + 128]
                    nc.tensor.matmul(
                        pp[:],
                        lhs,
                        emb[:, k * Vs + n * 500 : k * Vs + n * 500 + 500],
                        start=(k == 0),
                        stop=(k == NK - 1),
                    )
                st = p_stage.tile([128, 500], BF16, tag="pstage")
                if (m + n) % 2 == 0:
                    nc.scalar.copy(st[:], pp[:])
                else:
                    nc.vector.tensor_copy(st[:], pp[:])
                nc.sync.dma_start(
                    d_out[
                        :,
                        ci * Tc + NT * m : ci * Tc + NT * m + NT,
                        n * 500 : n * 500 + 500,
                    ].rearrange("b t v -> t b v"),
                    st[:],
                )

            for ci in range(NCH):
                gi_c = p_gi.tile([128, Tc, 768], BF16, tag="gic")
                nc.sync.dma_start(
                    gi_c[:], d_gis[ci * Tc : ci * Tc + Tc].rearrange("t p g -> p t g")
                )
                ht_c = p_ht.tile([128, NK, Tc * 32], BF16, tag="htc")
                for tl in range(Tc):
                    gh_rz = ps_rz.tile([128, 512], F32, tag="psrz")
                    gh_n = ps_n.tile([128, 256], F32, tag="psn")
                    # bias/gi injection (no h dependency; PE fills these early)
                    nc.tensor.matmul(
                        gh_rz[:], i128[:], gi_c[:, tl, 0:512], start=True, stop=False
                    )
                    nc.tensor.matmul(
                        gh_n[:], i128[:], bhn[:], start=True, stop=False
                    )
                    # r/z recurrence MMs first (sigmoid operand ready earliest)
                    for k in range(NK):
                        lhs = ht_prev(k)
                        for j in range(4):
                            nc.tensor.matmul(
                                gh_rz[32 * j : 32 * j + 32, :],
                                lhs,
                                whh[:, k * G3 + 768 * j : k * G3 + 768 * j + 512],
                                start=False,
                                stop=(k == NK - 1),
                                tile_position=(0, 32 * j),
                            )
                    # n MMs (overlap the sigmoid)
                    for k in range(NK):
                        lhs = ht_prev(k)
                        for j in range(4):
                            nc.tensor.matmul(
                                gh_n[32 * j : 32 * j + 32, :],
                                lhs,
                                whh[:, k * G3 + 768 * j + 512 : k * G3 + 768 * j + 768],
                                start=False,
                                stop=(k == NK - 1),
                                tile_position=(0, 32 * j),
                            )
                    # ---- gate math (bf16, partition-local) ----
                    rz = p_rz.tile([128, 512], BF16, tag="rz")
                    nc.scalar.activation(rz[:, 0:256], gh_rz[:, 0:256], AF.Sigmoid)
                    nc.scalar.activation(rz[:, 256:512], gh_rz[:, 256:512], AF.Sigmoid)
                    t12 = p_t12.tile([128, 512], BF16, tag="t12")
                    nc.vector.tensor_tensor(t12[:, 0:256], rz[:, 0:256], gh_n[:], ALU.mult)
                    nc.vector.tensor_tensor(
                        t12[:, 256:512], t12[:, 0:256], gi_c[:, tl, 512:768], ALU.add
                    )
                    nh = p_nh.tile([128, 256], BF16, tag="nh")
                    nc.scalar.activation(nh[:], t12[:, 256:512], AF.Tanh)
                    hdz = p_hdz.tile([128, 512], BF16, tag="hdz")
                    nc.vector.tensor_tensor(hdz[:, 0:256], h_prev, nh[:], ALU.subtract)
                    nc.vector.tensor_tensor(
                        hdz[:, 256:512], rz[:, 256:512], hdz[:, 0:256], ALU.mult
                    )
                    h_new = p_hb.tile([128, 256], BF16, tag="hb")
                    nc.vector.tensor_tensor(h_new[:], nh[:], hdz[:, 256:512], ALU.add)
                    # projection filler under the gate-math dependency chain
                    for _ in range(min(PPS, len(proj_queue))):
                        emit_proj_unit(*proj_queue.pop(0))
                    # ---- h'^T via identity-rhs matmuls, straight into HT ----
                    pT = ps_t.tile([128, 256], F32, tag="psht")
                    nc.tensor.matmul(
                        pT[:, 0:128], h_new[:, 0:128], i128[:], start=True, stop=True
                    )
                    nc.tensor.matmul(
                        pT[:, 128:256], h_new[:, 128:256], i128[:], start=True, stop=True
                    )
                    nc.vector.tensor_copy(
                        ht_c[:, :, tl * 32 : tl * 32 + 32],
                        pT[:].rearrange("p (h j b) -> p (h j) b", h=2, j=4),
                    )
                    ht_prev = (
                        lambda k, tl=tl, ht_c=ht_c: ht_c[
                            :, POS[k], tl * 32 : tl * 32 + 32
                        ]
                    )
                    h_prev = h_new[:]
                for m in range(NM):
                    for n in range(NV):
                        proj_queue.append((ci, ht_c, m, n))
            while proj_queue:
                emit_proj_unit(*proj_queue.pop(0))

    nc.finalize()
    _split_multi_waits(nc)
    return nc


def _gate_perm():
    P = np.empty(G3, np.int64)
    for j in range(4):
        u = np.arange(256) + 256 * j
        P[768 * j : 768 * j + 256] = u
        P[768 * j + 256 : 768 * j + 512] = H + u
        P[768 * j + 512 : 768 * j + 768] = 2 * H + u
    return P


def prep_inputs(enc_hiddens, emb_w, w_ih, w_hh, b_ih, b_hh, gold, T, Vs, n_cores):
    """Host-side shard + layout prep. Returns per-core input maps."""
    h0 = np.asarray(enc_hiddens, np.float32)[0]          # [B, H]
    emb_w = np.asarray(emb_w, np.float32)
    w_ih = np.asarray(w_ih, np.float32)
    w_hh = np.asarray(w_hh, np.float32)
    b_ih = np.asarray(b_ih, np.float32)
    b_hh = np.asarray(b_hh, np.float32)
    gold = np.asarray(gold)

    P = _gate_perm()
    whhp = _bf16(_kblock(w_hh[P].T))

    # teacher-forced inputs -> gate space (host prep; state-independent)
    idx = np.empty((T, B), np.int64)
    idx[0] = 1  # START_IDX
    if T > 1:
        idx[1:] = gold[:, : T - 1].T
    X = emb_w[idx].reshape(T * B, H)                      # [T*B, H]
    mask = (np.arange(G3) < 2 * H).astype(np.float32)
    gib_row = b_ih + b_hh * mask                          # rz biases summed; n: b_ih only
    GI = X @ w_ih.T + gib_row                             # [T*B, 3H] fp32
    gis = _bf16(
        np.ascontiguousarray(
            GI.reshape(T, B, 3, 4, 256).transpose(0, 3, 1, 2, 4).reshape(T, 128, 768)
        )
    )
    bhnp = _bf16(np.repeat(b_hh[2 * H :].reshape(4, 256), 32, axis=0))
    i128 = _bf16(np.eye(128, dtype=np.float32))
    h0b = _bf16(h0.reshape(B, 4, 256).transpose(1, 0, 2).reshape(128, 256))
    h0t = _bf16(
        np.ascontiguousarray(h0.reshape(B, 4, 2, 128).transpose(3, 2, 1, 0).reshape(128, 256))
    )
    embT = emb_w.T                                        # [H, V]
    maps = []
    for c in range(n_cores):
        embc = _bf16(_kblock(np.ascontiguousarray(embT[:, c * Vs : (c + 1) * Vs])))
        maps.append(
            dict(whhp=whhp, embc=embc, gis=gis, bhnp=bhnp, i128=i128, h0b=h0b, h0t=h0t)
        )
    return maps


_CACHE = {}


def run(enc_hiddens, emb_w, w_ih, w_hh, b_ih, b_hh, gold, T=256, Vs=4000,
        n_cores=8, Tc=8, trace=False):
    key = (T, Vs, n_cores, Tc)
    if key not in _CACHE:
        _CACHE[key] = build_program(T, Vs, Tc)
    nc = _CACHE[key]
    maps = prep_inputs(enc_hiddens, emb_w, w_ih, w_hh, b_ih, b_hh, gold, T, Vs, n_cores)
    res = run_bass_kernel_spmd(nc, maps, list(range(n_cores)), trace=trace)
    out = np.concatenate(
        [np.asarray(res.results[c]["scores"], np.float32) for c in range(n_cores)],
        axis=2,
    )
    return out, res


def kernel(enc_hiddens, emb_w, w_ih, w_hh, b_ih, b_hh, gold):
    out, _ = run(enc_hiddens, emb_w, w_ih, w_hh, b_ih, b_hh, gold)
    return out


# revision 26
# speedup vs baseline: 1.2097x; 1.0004x over previous
"""GRU decoder with tied-embedding projection on 8 Trainium2 NeuronCores.

Problem: B=32, T=256, H=1024, V=32000 (fp32).
    h_t = GRUCell(x_t, h_{t-1});  scores_t = h_t @ emb_w.T;  x_{t+1} = emb_w[gold_t]

Sharding: vocab-parallel (column-parallel tied projection). Every core runs the
(cheap, serial) GRU recurrence redundantly; each core computes a V/8 = 4000-wide
slice of the logits. No collectives; host concatenates the vocab slices.

Host-side input prep: the teacher-forced inputs are gathered (X = emb_w[idx])
and projected into gate space (GI = X @ w_ih.T + biases) on the host — GI is a
pure function of the inputs, independent of the recurrent state, and is laid
out per-step so the device reads it as a streamed operand.

Per-core kernel (all matmuls bf16, fp32 PSUM accumulation):
  - The recurrence matmul gh = h @ w_hh.T has only B=32 output rows, so it uses
    4-way PE *column tiling*: column group j computes a (permuted) 768-wide gate
    slice into PSUM partitions [32j, 32j+32).
  - Gate permutation P: group j holds [r,z,n] gates of hidden units
    [256j, 256j+256), so all gate math is partition-local.
  - gi_rz / the n-gate hidden bias are injected into PSUM with identity
    matmuls (PE is the only cross-partition data path), so the sigmoid reads
    PSUM directly.
  - All r/z MMs are emitted before the n MMs so the sigmoid's operand is ready
    at 2/3 of the recurrence span; gate math runs in bf16 (2x DVE modes).
  - h'^T is produced with identity-rhs matmuls straight into the chunk's HT
    tile, which doubles as the projection's stationary operand (no scatter).
  - Projection of chunk c-1 is interleaved between each step's recurrence MMs
    and the h-transpose, sized to cover the serial gate-math latency so the PE
    never idles (keeps the HAM clock-gate warm).
  - Logits are written in bf16 (host upcasts); well inside the error budget.
"""

import sys

import numpy as np

try:
    import concourse.bass as bass  # noqa: F401
except ImportError:  # grading env may not have it on sys.path
    sys.path.insert(0, "/opt/trn_rl_repo")

import concourse.bass as bass
import concourse.tile as tile
from concourse import mybir
from concourse.bass_utils import run_bass_kernel_spmd

import ml_dtypes

BF16 = mybir.dt.bfloat16
F32 = mybir.dt.float32
AF = mybir.ActivationFunctionType
ALU = mybir.AluOpType

N_CORES = 8
B = 32
H = 1024
NK = H // 128   # 8 k-tiles over the hidden dim
G3 = 3 * H      # 3072 gates
POS = [4 * (k % 2) + k // 2 for k in range(NK)]  # hid-block k -> HT col block


def _split_multi_waits(nc, limit=1):
    """Walrus (CoreV3, public build) accepts at most `limit` sem waits per
    instruction; move extra waits onto NoOps inserted just before."""
    n_new = 0
    for _name, bbw in nc.bb_map.items():
        insts = bbw.bb.instructions
        out, changed = [], False
        for inst in insts:
            si = inst.sync_info
            ws = list(si.on_wait) if si is not None else []
            if len(ws) > limit:
                changed = True
                for i in range(limit, len(ws), limit):
                    n_new += 1
                    nop = mybir.InstNoOp(
                        name=f"I-wsplit-{n_new}", engine=inst.engine, ins=[], outs=[]
                    )
                    nop.sync_info = mybir.SyncInfo(on_wait=ws[i : i + limit], on_update=[])
                    out.append(nop)
                inst.sync_info = mybir.SyncInfo(
                    on_wait=ws[:limit], on_update=list(si.on_update)
                )
            out.append(inst)
        if changed:
            bbw.bb.instructions = out
    return n_new


def _kblock(a):
    """[H, X] -> [128, NK*X]  (k-tile k occupies columns [k*X, (k+1)*X))."""
    hh, x = a.shape
    assert hh == H
    return np.ascontiguousarray(a.reshape(NK, 128, x).transpose(1, 0, 2).reshape(128, NK * x))


def _bf16(a):
    return np.asarray(a, dtype=ml_dtypes.bfloat16)


def build_program(T, Vs, Tc, PPS=2):
    """Build the SPMD bass program (identical on all cores)."""
    assert T % Tc == 0
    NCH = T // Tc            # chunks
    NV = Vs // 500           # 500-wide vocab chunks
    NM = (Tc * B) // 128     # projection m-tiles per chunk
    NT = 128 // B            # steps per projection m-tile (m covers tl in [NT*m, NT*m+NT))

    nc = bass.Bass()
    d_whh = nc.declare_dram_parameter("whhp", [128, NK * G3], BF16, isOutput=False)
    d_emb = nc.declare_dram_parameter("embc", [128, NK * Vs], BF16, isOutput=False)
    d_gis = nc.declare_dram_parameter("gis", [T, 128, 768], BF16, isOutput=False)
    d_bhn = nc.declare_dram_parameter("bhnp", [128, 256], BF16, isOutput=False)
    d_i128 = nc.declare_dram_parameter("i128", [128, 128], BF16, isOutput=False)
    d_h0b = nc.declare_dram_parameter("h0b", [128, 256], BF16, isOutput=False)
    d_h0t = nc.declare_dram_parameter("h0t", [128, 256], BF16, isOutput=False)
    d_out = nc.declare_dram_parameter("scores", [B, T, Vs], BF16, isOutput=True)

    with tile.TileContext(nc) as tc:
        with (
            tc.tile_pool(name="consts", bufs=1) as consts,
            tc.tile_pool(name="gic", bufs=2) as p_gi,
            tc.tile_pool(name="htc", bufs=3) as p_ht,
            tc.tile_pool(name="rz", bufs=2) as p_rz,
            tc.tile_pool(name="t12", bufs=2) as p_t12,
            tc.tile_pool(name="nh", bufs=2) as p_nh,
            tc.tile_pool(name="hdz", bufs=2) as p_hdz,
            tc.tile_pool(name="hb", bufs=2) as p_hb,
            tc.tile_pool(name="pstage", bufs=4) as p_stage,
            tc.tile_pool(name="psrz", bufs=2, space="PSUM") as ps_rz,
            tc.tile_pool(name="psn", bufs=2, space="PSUM") as ps_n,
            tc.tile_pool(name="psht", bufs=1, space="PSUM") as ps_t,
            tc.tile_pool(name="pspr", bufs=3, space="PSUM") as ps_pr,
        ):
            whh = consts.tile([128, NK * G3], BF16, tag="whh")
            nc.sync.dma_start(whh[:], d_whh[:])
            i128 = consts.tile([128, 128], BF16, tag="i128")
            nc.sync.dma_start(i128[:], d_i128[:])
            bhn = consts.tile([128, 256], BF16, tag="bhn")
            nc.sync.dma_start(bhn[:], d_bhn[:])
            h0b = consts.tile([128, 256], BF16, tag="h0b")
            nc.sync.dma_start(h0b[:], d_h0b[:])
            h0t = consts.tile([128, 256], BF16, tag="h0t")
            nc.sync.dma_start(h0t[:], d_h0t[:])
            emb = consts.tile([128, NK * Vs], BF16, tag="emb")
            nc.sync.dma_start(emb[:], d_emb[:])

            h_prev = h0b[:]            # [32j+b, 256]  gate-math layout
            ht_prev = lambda k: h0t[:, 32 * POS[k] : 32 * POS[k] + 32]
            proj_queue = []

            def emit_proj_unit(ci, ht_c, m, n):
                # HT chunk layout: col = pos*(Tc*32) + tl*32 + b; m-tile m is
                # rows (tl, b) for tl in [NT*m, NT*m+NT) -> contiguous lhsT.
                pp = ps_pr.tile([128, 500], F32, tag="pspr")
                for k in range(NK):
                    lhs = ht_c[:, POS[k], 32 * NT * m : 32 * NT * m + 128]
                    nc.tensor.matmul(
                        pp[:],
                        lhs,
                        emb[:, k * Vs + n * 500 : k * Vs + n * 500 + 500],
                        start=(k == 0),
                        stop=(k == NK - 1),
                    )
                st = p_stage.tile([128, 500], BF16, tag="pstage")
                if (m + n) % 2 == 0:
                    nc.scalar.copy(st[:], pp[:])
                else:
                    nc.vector.tensor_copy(st[:], pp[:])
                nc.sync.dma_start(
                    d_out[
                        :,
                        ci * Tc + NT * m : ci * Tc + NT * m + NT,
                        n * 500 : n * 500 + 500,
                    ].rearrange("b t v -> t b v"),
                    st[:],
                )

            for ci in range(NCH):
                gi_c = p_gi.tile([128, Tc, 768], BF16, tag="gic")
                nc.sync.dma_start(
                    gi_c[:], d_gis[ci * Tc : ci * Tc + Tc].rearrange("t p g -> p t g")
                )
                ht_c = p_ht.tile([128, NK, Tc * 32], BF16, tag="htc")
                for tl in range(Tc):
                    gh_rz = ps_rz.tile([128, 512], F32, tag="psrz")
                    gh_n = ps_n.tile([128, 256], F32, tag="psn")
                    # bias/gi injection (no h dependency; PE fills these early)
                    nc.tensor.matmul(
                        gh_rz[:], i128[:], gi_c[:, tl, 0:512], start=True, stop=False
                    )
                    nc.tensor.matmul(
                        gh_n[:], i128[:], bhn[:], start=True, stop=False
                    )
                    # r/z recurrence MMs first (sigmoid operand ready earliest)
                    for k in range(NK):
                        lhs = ht_prev(k)
                        for j in range(4):
                            nc.tensor.matmul(
                                gh_rz[32 * j : 32 * j + 32, :],
                                lhs,
                                whh[:, k * G3 + 768 * j : k * G3 + 768 * j + 512],
                                start=False,
                                stop=(k == NK - 1),
                                tile_position=(0, 32 * j),
                            )
                    # n MMs (overlap the sigmoid)
                    for k in range(NK):
                        lhs = ht_prev(k)
                        for j in range(4):
                            nc.tensor.matmul(
                                gh_n[32 * j : 32 * j + 32, :],
                                lhs,
                                whh[:, k * G3 + 768 * j + 512 : k * G3 + 768 * j + 768],
                                start=False,
                                stop=(k == NK - 1),
                                tile_position=(0, 32 * j),
                            )
                    # ---- gate math (bf16, partition-local) ----
                    rz = p_rz.tile([128, 512], BF16, tag="rz")
                    nc.scalar.activation(rz[:, 0:256], gh_rz[:, 0:256], AF.Sigmoid)
                    nc.scalar.activation(rz[:, 256:512], gh_rz[:, 256:512], AF.Sigmoid)
                    t12 = p_t12.tile([128, 512], BF16, tag="t12")
                    nc.vector.tensor_tensor(t12[:, 0:256], rz[:, 0:256], gh_n[:], ALU.mult)
                    nc.vector.tensor_tensor(
                        t12[:, 256:512], t12[:, 0:256], gi_c[:, tl, 512:768], ALU.add
                    )
                    nh = p_nh.tile([128, 256], BF16, tag="nh")
                    nc.scalar.activation(nh[:], t12[:, 256:512], AF.Tanh)
                    hdz = p_hdz.tile([128, 512], BF16, tag="hdz")
                    nc.vector.tensor_tensor(hdz[:, 0:256], h_prev, nh[:], ALU.subtract)
                    nc.vector.tensor_tensor(
                        hdz[:, 256:512], rz[:, 256:512], hdz[:, 0:256], ALU.mult
                    )
                    h_new = p_hb.tile([128, 256], BF16, tag="hb")
                    nc.vector.tensor_tensor(h_new[:], nh[:], hdz[:, 256:512], ALU.add)
                    # projection filler under the gate-math dependency chain
                    for _ in range(min(PPS, len(proj_queue))):
                        emit_proj_unit(*proj_queue.pop(0))
                    # ---- h'^T via identity-rhs matmuls, straight into HT ----
                    pT = ps_t.tile([128, 256], F32, tag="psht")
                    nc.tensor.matmul(
                        pT[:, 0:128], h_new[:, 0:128], i128[:], start=True, stop=True
                    )
                    nc.tensor.matmul(
                        pT[:, 128:256], h_new[:, 128:256], i128[:], start=True, stop=True
                    )
                    nc.vector.tensor_copy(
                        ht_c[:, :, tl * 32 : tl * 32 + 32],
                        pT[:].rearrange("p (h j b) -> p (h j) b", h=2, j=4),
                    )
                    ht_prev = (
                        lambda k, tl=tl, ht_c=ht_c: ht_c[
                            :, POS[k], tl * 32 : tl * 32 + 32
                        ]
                    )
                    h_prev = h_new[:]
                for m in range(NM):
                    for n in range(NV):
                        proj_queue.append((ci, ht_c, m, n))
            while proj_queue:
                emit_proj_unit(*proj_queue.pop(0))

    nc.finalize()
    _split_multi_waits(nc)
    return nc


def _gate_perm():
    P = np.empty(G3, np.int64)
    for j in range(4):
        u = np.arange(256) + 256 * j
        P[768 * j : 768 * j + 256] = u
        P[768 * j + 256 : 768 * j + 512] = H + u
        P[768 * j + 512 : 768 * j + 768] = 2 * H + u
    return P


def prep_inputs(enc_hiddens, emb_w, w_ih, w_hh, b_ih, b_hh, gold, T, Vs, n_cores):
    """Host-side shard + layout prep. Returns per-core input maps."""
    h0 = np.asarray(enc_hiddens, np.float32)[0]          # [B, H]
    emb_w = np.asarray(emb_w, np.float32)
    w_ih = np.asarray(w_ih, np.float32)
    w_hh = np.asarray(w_hh, np.float32)
    b_ih = np.asarray(b_ih, np.float32)
    b_hh = np.asarray(b_hh, np.float32)
    gold = np.asarray(gold)

    P = _gate_perm()
    whhp = _bf16(_kblock(w_hh[P].T))

    # teacher-forced inputs -> gate space (host prep; state-independent)
    idx = np.empty((T, B), np.int64)
    idx[0] = 1  # START_IDX
    if T > 1:
        idx[1:] = gold[:, : T - 1].T
    X = emb_w[idx].reshape(T * B, H)                      # [T*B, H]
    mask = (np.arange(G3) < 2 * H).astype(np.float32)
    gib_row = b_ih + b_hh * mask                          # rz biases summed; n: b_ih only
    GI = X @ w_ih.T + gib_row                             # [T*B, 3H] fp32
    gis = _bf16(
        np.ascontiguousarray(
            GI.reshape(T, B, 3, 4, 256).transpose(0, 3, 1, 2, 4).reshape(T, 128, 768)
        )
    )
    bhnp = _bf16(np.repeat(b_hh[2 * H :].reshape(4, 256), 32, axis=0))
    i128 = _bf16(np.eye(128, dtype=np.float32))
    h0b = _bf16(h0.reshape(B, 4, 256).transpose(1, 0, 2).reshape(128, 256))
    h0t = _bf16(
        np.ascontiguousarray(h0.reshape(B, 4, 2, 128).transpose(3, 2, 1, 0).reshape(128, 256))
    )
    embT = emb_w.T                                        # [H, V]
    maps = []
    for c in range(n_cores):
        embc = _bf16(_kblock(np.ascontiguousarray(embT[:, c * Vs : (c + 1) * Vs])))
        maps.append(
            dict(whhp=whhp, embc=embc, gis=gis, bhnp=bhnp, i128=i128, h0b=h0b, h0t=h0t)
        )
    return maps


_CACHE = {}


def run(enc_hiddens, emb_w, w_ih, w_hh, b_ih, b_hh, gold, T=256, Vs=4000,
        n_cores=8, Tc=8, trace=False):
    key = (T, Vs, n_cores, Tc)
    if key not in _CACHE:
        _CACHE[key] = build_program(T, Vs, Tc)
    nc = _CACHE[key]
    maps = prep_inputs(enc_hiddens, emb_w, w_ih, w_hh, b_ih, b_hh, gold, T, Vs, n_cores)
    res = run_bass_kernel_spmd(nc, maps, list(range(n_cores)), trace=trace)
    out = np.concatenate(
        [np.asarray(res.results[c]["scores"], np.float32) for c in range(n_cores)],
        axis=2,
    )
    return out, res


def kernel(enc_hiddens, emb_w, w_ih, w_hh, b_ih, b_hh, gold):
    out, _ = run(enc_hiddens, emb_w, w_ih, w_hh, b_ih, b_hh, gold)
    return out
